# revision 22
# baseline (speedup 1.0000x reference)
"""Multi-head self-attention Trainium2 kernel.

Problem: B=2, N=2048, D=1024, H=16 heads (HD=64), fp32 I/O.

Sharding (8 cores): core c handles batch b = c//4 and the 4-head group
g = c%4 (data parallel on B, tensor parallel on heads).  Each core:
  1. QKV projection for its 768 columns (q cols pre-scaled by HD^-0.5),
     producing qT/kT channel-major and V row-major augmented with a
     ones column (softmax denominator rides the PV matmul as row 64).
  2. Transposed attention, two heads packed per pass (head A in PE rows
     0-63, head B in rows 64-127 -> concurrent row-group matmuls):
     S^T scores in PSUM, one exp per m-tile on ScalarE (no max
     subtraction -- logits are O(1) here), PV contracting over m.
  3. Normalization: reciprocal of the denominator row, broadcast via a
     K=1 matmul, multiply into outT.
  4. Output projection against its 256 rows of w_proj -> bf16 partial.
Host sums the 4 partials per batch and adds the bias terms.

Schedule: the kernel is jointly limited by ACT (128 exps of [128,1024],
~1.15us each, ~147us total) and the PE (~137us of matmul streams), so
the emission is paced to keep both near-busy from t~10us on:
  - DMA priority order (3 queues): pair-0 q/k weight cols + xt first
    half + v weights (ramp set, ~3MB), then xt second half, pair-1 q/k
    cols, w_proj.  The prologue computes exactly what quarter 0 needs
    first (kT m0-511, qT n0-511, V m0-255) paced by the arriving DMAs,
    so the first exp lands ~landmark 10-11us.
  - All remaining qT/kT column blocks, V groups, PV accumulations and
    output-projection blocks are emitted through a deadline/budget
    pacer: each is released only when the modeled PE backlog is below
    the modeled ACT backlog (so the in-order PE queue never delays the
    scores->exp chain), or when its deadline forces it.
  - PV runs up to 4 iterations behind exp (eb ring depth 6), which
    spreads quarter-0's forced V work into quarter 1 and carries each
    quarter's PV drain into the next quarter (no boundary stall).
PSUM (8 banks, all used): scores 2x[128,1024] (4), PV accum 2x[65,512]
(2), one [128,512] ring for qk/proj generations (1), one [128,512]
ring for V-pair generations + epilogue broadcasts (1).
"""

import numpy as np
import ml_dtypes

B, N, D, H = 2, 2048, 1024, 16
HD = D // H  # 64
SCALE = HD ** -0.5
NCORES = 8
HPC = H // 4  # heads per core
CPC = HPC * HD  # channels per core = 256
P = 128
DT = D // P  # 8 contraction tiles
NT = N // P  # 16 sequence tiles

_CACHE = {}

# pacer cost constants (ns, approximate)
C_SC = 230      # score pair (row-group concurrent)
C_EXP = 1150    # exp [128,1024]
C_PV = 450      # both heads' PV matmuls for one m-tile
C_QKC = 1000     # one 4-matmul qk chunk (half a [P,512] generation)
C_VG = 1800     # one packed V generation (2 m-tiles, 16 matmuls)
C_EPI = 450     # epilogue broadcast matmuls
C_PRJ = 560     # one [P,512] proj generation (2 matmuls)
SLACK = 700
PVLAG = 4       # max PV iterations behind exp (ebpool bufs - 2)


def build_nc():
    import concourse.tile as tile
    from concourse import bacc, mybir

    nc = bacc.Bacc("TRN2", target_bir_lowering=False, debug=False,
                   num_devices=NCORES)
    bf16 = mybir.dt.bfloat16
    xt = nc.dram_tensor("xt", [P, 4, DT * 512], bf16,
                        kind="ExternalInput").ap()
    w = nc.dram_tensor("w", [P, 3, DT * 256], bf16,
                       kind="ExternalInput").ap()
    wp = nc.dram_tensor("wp", [P, 2, D], bf16, kind="ExternalInput").ap()
    y = nc.dram_tensor("y", [N, D], bf16, kind="ExternalOutput").ap()

    with tile.TileContext(nc) as tc:
        _mha_tile_kernel(tc, y, xt, w, wp)
    nc.compile()
    return nc


def _mha_tile_kernel(tc, y, xt, w, wp):
    from contextlib import ExitStack
    from concourse import mybir

    nc = tc.nc
    bf16 = mybir.dt.bfloat16
    f32 = mybir.dt.float32
    EXP = mybir.ActivationFunctionType.Exp
    # qk_sb slot -> (w block, col offset); host w blocks [q01|k01], [v],
    # [q23|k23], each [P, DT, 256]
    QKCOL = {0: (0, 0), 2: (0, 128), 1: (2, 0), 3: (2, 128)}

    with ExitStack() as ctx:
        consts = ctx.enter_context(tc.tile_pool(name="consts", bufs=1))
        work = ctx.enter_context(tc.tile_pool(name="work", bufs=1))
        ebpool = ctx.enter_context(tc.tile_pool(name="eb", bufs=6))
        ypool = ctx.enter_context(tc.tile_pool(name="yp", bufs=4))
        rpool = ctx.enter_context(tc.tile_pool(name="rp", bufs=8))
        pvspool = ctx.enter_context(tc.tile_pool(name="pvs", bufs=6))
        ps_sc = ctx.enter_context(
            tc.tile_pool(name="ps_sc", bufs=2, space="PSUM"))   # 4 banks
        ps_pv = ctx.enter_context(
            tc.tile_pool(name="ps_pv", bufs=2, space="PSUM"))   # 2 banks
        ps_flex = ctx.enter_context(
            tc.tile_pool(name="ps_flex", bufs=2, space="PSUM"))  # 2 banks

        # ---- input DMA: few large strided transfers (dispatch costs
        # ~650ns/queue and each queue ring only allows 4 outstanding, so
        # many small DMAs serialize the ramp).  Priority: pair-0 q/k cols
        # + v cols + xt cols 0-511 (everything quarter 0 mt0-3 needs),
        # then xt 512-1023, xt second half, pair-1 q/k cols, w_proj.
        w_sb = work.tile([P, 3, DT * 256], bf16, tag="w")
        xt_sb = work.tile([P, 4, DT * 512], bf16, tag="xt")
        wp_sb = work.tile([P, 2, D], bf16, tag="wp")
        # Inputs are host-packed in SBUF layout (partition-major), so each
        # transfer is one DMA with multi-KB contiguous lines at full wire
        # rate.  Priority: pair-0 q/k cols, xt n0-511, v cols (the ramp
        # set, 2MB), xt n512-1023, xt second half, pair-1 q/k cols, wp.
        # every transfer is a plain 2D contiguous slice ([128 x <=4KB
        # lines]); 3D+ slice patterns both transfer slowly and mis-order
        # against their completion semaphores on hardware
        H2 = DT * 512 // 2
        # sweep-critical set first on all three queues (w qk01+v, xt block
        # 0): ~2MB of wire before the prologue can finish; everything else
        # strictly after
        nc.sync.dma_start(w_sb[:, 0], w[:, 0])            # q01|k01 cols
        nc.gpsimd.dma_start(xt_sb[:, 0, 0:H2], xt[:, 0, 0:H2])
        nc.scalar.dma_start(xt_sb[:, 0, H2:2 * H2], xt[:, 0, H2:2 * H2])
        nc.sync.dma_start(w_sb[:, 1], w[:, 1])            # v cols
        nc.gpsimd.dma_start(xt_sb[:, 1, 0:H2], xt[:, 1, 0:H2])
        nc.scalar.dma_start(xt_sb[:, 1, H2:2 * H2], xt[:, 1, H2:2 * H2])
        nc.gpsimd.dma_start(xt_sb[:, 2, 0:H2], xt[:, 2, 0:H2])
        nc.scalar.dma_start(xt_sb[:, 2, H2:2 * H2], xt[:, 2, H2:2 * H2])
        nc.gpsimd.dma_start(xt_sb[:, 3, 0:H2], xt[:, 3, 0:H2])
        nc.scalar.dma_start(xt_sb[:, 3, H2:2 * H2], xt[:, 3, H2:2 * H2])
        nc.sync.dma_start(w_sb[:, 2], w[:, 2])            # q23|k23 cols
        nc.sync.dma_start(wp_sb[:, 0], wp[:, 0])
        nc.sync.dma_start(wp_sb[:, 1], wp[:, 1])

        ones_sb = consts.tile([1, N], bf16, tag="ones")
        nc.vector.memset(ones_sb, 1.0)
        qk_sb = work.tile([P, 4, N], bf16, tag="qk")
        vaug_sb = work.tile([P, NT, HPC, HD + 1], bf16, tag="vaug")
        nc.vector.memset(vaug_sb[:, :, :, HD:HD + 1], 1.0)
        outT_sb = work.tile([P, 2, N], bf16, tag="outT")

        # warm the Exp table during the DMA ramp (one-time 1.3us load)
        wrm = consts.tile([1, 1], f32, tag="wrm")
        nc.vector.memset(wrm, 0.0)
        nc.scalar.activation(out=wrm, in_=wrm, func=EXP)

        # ---- pacer state ----
        pe_t = [0.0]
        act_t = [0.0]

        # ---- prologue: kT m0-511 / qT n0-511 / V m0-255, DMA-paced ----
        pro_k = ps_flex.tile([P, 512], f32, tag="flex", name="prok")
        pro_q = ps_sc.tile([P, 512], f32, tag="sc", name="proq")
        v01 = ps_flex.tile([P, 2, CPC], f32, tag="flex", name="v01")
        for dt in range(DT):
            st, sp = dt == 0, dt == DT - 1
            nc.tensor.matmul(
                pro_k, lhsT=w_sb[:, 0, dt * 256 + 128:dt * 256 + 256],
                rhs=xt_sb[:, 0, dt * 512:(dt + 1) * 512], start=st, stop=sp)
            nc.tensor.matmul(
                pro_q, lhsT=w_sb[:, 0, dt * 256:dt * 256 + 128],
                rhs=xt_sb[:, 0, dt * 512:(dt + 1) * 512], start=st, stop=sp)
            # v01[:, 0] is its own accumulation group in its bank; it may
            # interleave with the other banks' groups above but not with
            # the v01[:, 1] group (one pending group per bank), which is
            # deferred to a deadline-0 filler below.
            nc.tensor.matmul(
                v01[:, 0], lhsT=xt_sb[:, 0, dt * 512:dt * 512 + P],
                rhs=w_sb[:, 1, dt * 256:(dt + 1) * 256], start=st, stop=sp)
        nc.scalar.copy(out=qk_sb[:, 2, 0:512], in_=pro_k)
        nc.scalar.copy(out=qk_sb[:, 0, 0:512], in_=pro_q)

        def v01b():
            for dt in range(DT):
                nc.tensor.matmul(
                    v01[:, 1], lhsT=xt_sb[:, 0, dt * 512 + P:dt * 512 + 2 * P],
                    rhs=w_sb[:, 1, dt * 256:(dt + 1) * 256],
                    start=(dt == 0), stop=(dt == DT - 1))
            nc.vector.tensor_copy(
                out=vaug_sb[:, 0:2, :, 0:HD],
                in_=v01.rearrange("p two (h d) -> p two h d", h=HPC))

        # ---- qk generations ([P,512] psum on the qkf ring) ----
        def qk_gen_chunks(slot, j, copy_eng=None):
            state = {}

            def emit(dts, last):
                if "ps" not in state:
                    state["ps"] = ps_flex.tile([P, 512], f32, tag="flex",
                                              name=f"qk{slot}{j}")
                ps = state["ps"]
                blk, c0 = QKCOL[slot]
                for dt in dts:
                    nc.tensor.matmul(
                        ps, lhsT=w_sb[:, blk, dt * 256 + c0:dt * 256 + c0 + P],
                        rhs=xt_sb[:, j, dt * 512:(dt + 1) * 512],
                        start=(dt == 0), stop=(dt == DT - 1))
                if last:
                    dst = qk_sb[:, slot, j * 512:(j + 1) * 512]
                    if copy_eng is nc.scalar:
                        nc.scalar.copy(out=dst, in_=ps)
                    else:
                        nc.vector.tensor_copy(out=dst, in_=ps)

            return [lambda: emit(range(0, 4), False),
                    lambda: emit(range(4, DT), True)]

        # static fillers: (deadline git, cost, fn)
        fillers = []

        def add_gen(slot, j, d0, d1, copy_eng=None):
            a, b = qk_gen_chunks(slot, j, copy_eng)
            fillers.append([d0, C_QKC, a])
            fillers.append([d1, C_QKC, b])

        fillers.append([0, 950, v01b])   # V m128-255 (need before PV pops)
        add_gen(2, 1, 1, 2)      # kT m512-1023 (need git 4)
        add_gen(2, 2, 5, 6)      # kT m1024-1535 (need git 8)
        add_gen(2, 3, 9, 10)      # kT m1536-2047 (need git 12)
        add_gen(0, 1, 11, 13)    # qT n512-1023  (need git 16)
        add_gen(0, 2, 25, 27)    # qT n1024-1535 (need git 32)
        add_gen(0, 3, 41, 43)    # qT n1536-2047 (need git 48)
        add_gen(3, 0, 50, 52)    # kT p1 m0-511  (need git 64)
        add_gen(1, 0, 54, 56)    # qT p1 n0-511  (need git 64)
        add_gen(3, 1, 60, 62)    # kT p1 m512-1023 (need git 68)
        add_gen(3, 2, 64, 66)    # kT p1 m1024-1535 (need git 72)
        add_gen(3, 3, 70, 71)    # kT p1 m1536-2047 (need git 76)
        add_gen(1, 1, 74, 76)    # qT p1 n512-1023 (need git 80)
        add_gen(1, 2, 88, 90)    # qT p1 n1024-1535 (need git 96)
        add_gen(1, 3, 104, 106)  # qT p1 n1536-2047 (need git 112)
        fil_i = [0]

        # ---- V generations (packed 2 m-tiles, vps ring) ----
        v_next = [2]   # m-tiles 0,1 done in the prologue

        def emit_v_gen():
            mt = v_next[0]
            v_next[0] += 2
            ps = ps_flex.tile([P, 2, CPC], f32, tag="flex", name=f"v{mt}")
            for k2 in range(2):
                mtk = mt + k2
                o = (mtk % 4) * P
                for dt in range(DT):
                    nc.tensor.matmul(
                        ps[:, k2],
                        lhsT=xt_sb[:, mtk // 4, dt * 512 + o:dt * 512 + o + P],
                        rhs=w_sb[:, 1, dt * 256:(dt + 1) * 256],
                        start=(dt == 0), stop=(dt == DT - 1))
            nc.vector.tensor_copy(
                out=vaug_sb[:, mt:mt + 2, :, 0:HD],
                in_=ps.rearrange("p two (h d) -> p two h d", h=HPC))
            pe_t[0] += C_VG

        # ---- PV pipeline (lag-K behind exp), quarter finish ----
        from collections import deque
        pvq = deque()            # (git, pair, q, mt, eb)
        qpv = {}
        pending = []             # staged epilogue callables
        cur_git = [0]
        qfin = [-10]             # git of the last quarter-finish

        def finish_quarter(pair, q, pv):
            qfin[0] = cur_git[0]
            last = pair == 1 and q == 3
            pvs = [pvspool.tile([HD + 1, 512], f32, tag="pvs",
                                name=f"pvs{pair}{q}{i}") for i in range(2)]
            dcps = [rpool.tile([1, 512], f32, tag="dcp",
                               name=f"dcp{pair}{q}{i}") for i in range(2)]
            for i in range(2):
                # the final quarter splits across DVE+ACT for minimum tail
                # (GPSIMD cannot access PSUM, so DVE carries the rest);
                # the denominator row moves to partition 0 for the DVE
                # reciprocal ucode, which needs a partition-0 operand
                if last:
                    nc.scalar.copy(out=pvs[i], in_=pv[i])
                    nc.scalar.copy(out=dcps[i], in_=pv[i][HD:HD + 1, :])
                else:
                    nc.vector.tensor_copy(out=pvs[i], in_=pv[i])
                    nc.vector.tensor_copy(out=dcps[i],
                                          in_=pv[i][HD:HD + 1, :])
            rbfs = [rpool.tile([1, 512], bf16, tag="rbf",
                               name=f"rbf{pair}{q}{i}") for i in range(2)]

            def recs():
                for i in range(2):
                    rec = rpool.tile([1, 512], f32, tag="rec",
                                     name=f"rec{pair}{q}{i}")
                    nc.vector.reciprocal_approx_fast(out=rec, in_=dcps[i])
                    nc.vector.tensor_copy(out=rbfs[i], in_=rec)

            def epi():
                n0 = q * 512
                for i in range(2):
                    bp = i * HD
                    bc = ps_flex.tile([HD, 512], f32, tag="flex",
                                     name=f"bc{pair}{q}{i}")
                    nc.tensor.matmul(bc, lhsT=ones_sb[:, 0:HD], rhs=rbfs[i],
                                     start=True, stop=True)
                    nc.vector.tensor_mul(
                        out=outT_sb[bp:bp + HD, pair, n0:n0 + 512],
                        in0=bc, in1=pvs[i][0:HD, :])
                pe_t[0] += C_EPI
                if pair == 1 and q < 3:
                    for nt in range(4 * q, 4 * q + 4):
                        add_proj(nt)

            pending.append(recs)
            pending.append(epi)

        def pop_pv():
            _, pair, q, mt, eb = pvq.popleft()
            if pair == 0 and v_next[0] <= mt:
                emit_v_gen()
            if mt == 0:
                qpv[(pair, q)] = [
                    ps_pv.tile([HD + 1, 512], f32, tag="pv",
                               name=f"pv{pair}{q}{i}") for i in range(2)]
            pv = qpv[(pair, q)]
            for i in range(2):
                nc.tensor.matmul(pv[i],
                                 lhsT=vaug_sb[:, mt, 2 * pair + i, :],
                                 rhs=eb[:, i * 512:(i + 1) * 512],
                                 start=(mt == 0), stop=(mt == NT - 1))
            pe_t[0] += C_PV
            if mt == NT - 1:
                finish_quarter(pair, q, pv)

        # ---- output projection generations (qkf ring) ----
        in_tail = [False]

        def add_proj(nt):
            yt = ypool.tile([P, D], bf16, tag="y", name=f"y{nt}")

            def half(hf):
                ps = ps_flex.tile([P, 512], f32, tag="flex", name=f"pj{nt}{hf}")
                for ct in range(2):
                    nc.tensor.matmul(
                        ps, lhsT=outT_sb[:, ct, nt * P:(nt + 1) * P],
                        rhs=wp_sb[:, ct, hf * 512:(hf + 1) * 512],
                        start=(ct == 0), stop=(ct == 1))
                # post-loop, ACT is idle: split the psum release with DVE
                ce = nc.scalar if (in_tail[0] and hf == 0) else nc.vector
                if ce is nc.scalar:
                    ce.copy(out=yt[:, hf * 512:(hf + 1) * 512], in_=ps)
                else:
                    ce.tensor_copy(out=yt[:, hf * 512:(hf + 1) * 512], in_=ps)
                if hf == 1:
                    eng = nc.sync if nt % 2 == 0 else nc.gpsimd
                    eng.dma_start(y[nt * P:(nt + 1) * P, :], yt)

            fillers.append([995, C_PRJ, lambda: half(0)])
            fillers.append([996, C_PRJ, lambda: half(1)])

        # ---- main attention loop ----
        for pair in range(2):
            for q in range(4):
                n0 = q * 512
                for mt in range(NT):
                    git = pair * 64 + q * 16 + mt
                    cur_git[0] = git
                    ps = ps_sc.tile([P, 1024], f32, tag="sc")
                    for i in range(2):
                        bp = i * HD
                        nc.tensor.matmul(
                            ps[:, i * 512:(i + 1) * 512],
                            lhsT=qk_sb[bp:bp + HD, 2 + pair,
                                       mt * P:(mt + 1) * P],
                            rhs=qk_sb[bp:bp + HD, pair, n0:n0 + 512],
                            start=True, stop=True)
                    pe_t[0] += C_SC
                    eb = ebpool.tile([P, 1024], bf16, tag="eb")
                    nc.scalar.activation(out=eb, in_=ps, func=EXP)
                    act_t[0] += C_EXP
                    pvq.append((git, pair, q, mt, eb))
                    # forced: PV lag cap, filler deadlines; at the very
                    # end run PV nearly caught-up so the tail chain is short
                    while len(pvq) > (PVLAG if git < 124 else 1):
                        pop_pv()
                    while fil_i[0] < len(fillers) and \
                            fillers[fil_i[0]][0] <= git:
                        c = fillers[fil_i[0]]
                        fil_i[0] += 1
                        c[2]()
                        pe_t[0] += c[1]
                    if pending and mt % 2 == 0:
                        pending.pop(0)()
                    # budget: release work while the PE backlog trails ACT
                    # (max 2 filler generations per iteration to avoid psum
                    # ring bursts; PV pops need a 2-iteration-old exp and 2
                    # iterations of spacing after a quarter release)
                    nfil = 0
                    while pe_t[0] < act_t[0] - SLACK:
                        if nfil < 1 and fil_i[0] < len(fillers) and \
                                fillers[fil_i[0]][0] <= git + PVLAG and \
                                not (git >= 120 and
                                     fillers[fil_i[0]][0] >= 900):
                            c = fillers[fil_i[0]]
                            fil_i[0] += 1
                            c[2]()
                            pe_t[0] += c[1]
                            nfil += 1
                        elif pvq and git - pvq[0][0] >= 2 and \
                                git >= qfin[0] + 2:
                            pop_pv()
                        elif nfil < 1 and fil_i[0] < len(fillers) and \
                                not (git >= 120 and
                                     fillers[fil_i[0]][0] >= 900):
                            c = fillers[fil_i[0]]
                            fil_i[0] += 1
                            c[2]()
                            pe_t[0] += c[1]
                            nfil += 1
                        else:
                            break

        # ---- tail: drain PV pipeline, last epilogue, last projections ----
        in_tail[0] = True
        while pvq:
            pop_pv()
        for fn in pending:
            fn()
        while fil_i[0] < len(fillers):
            c = fillers[fil_i[0]]
            fil_i[0] += 1
            c[2]()
        for nt in range(12, 16):
            ps = ps_sc.tile([P, 1024], f32, tag="sc", name=f"pjt{nt}")
            for ct in range(2):
                for ec in range(2):
                    nc.tensor.matmul(
                        ps[:, ec * 512:(ec + 1) * 512],
                        lhsT=outT_sb[:, ct, nt * P:(nt + 1) * P],
                        rhs=wp_sb[:, ct, ec * 512:(ec + 1) * 512],
                        start=(ct == 0), stop=(ct == 1))
            yt = ypool.tile([P, D], bf16, tag="y", name=f"yt{nt}")
            nc.scalar.copy(out=yt[:, 0:512], in_=ps[:, 0:512])
            nc.vector.tensor_copy(out=yt[:, 512:1024], in_=ps[:, 512:1024])
            eng = (nc.sync, nc.gpsimd, nc.scalar, nc.sync)[nt - 12]
            eng.dma_start(y[nt * P:(nt + 1) * P, :], yt)


def make_in_maps(x, w_qkv, b_qkv, w_proj):
    """Build the 8 per-core input dicts (host-side sharding).

    Biases are not sent to the device: b_k shifts every logit in a
    softmax row by the same amount (cancels exactly), b_v shifts the
    attention output by a constant (folded into y on the host as
    b_v @ w_proj), and b_q is zero for this problem (kernel() falls
    back to an exact host path if it ever is not).

    All inputs are packed in the exact SBUF layout (partition-major) so
    each tensor loads as one or few DMAs with multi-KB contiguous lines:
    xt [P, 4 n-blocks, DT, 512], w [P, 3 blocks ([q01|k01], [v],
    [q23|k23]), DT, 256], wp [P, 2, D].
    """
    bf = ml_dtypes.bfloat16
    x = np.asarray(x, np.float32)
    w_qkv = np.asarray(w_qkv, np.float32)
    w_proj = np.asarray(w_proj, np.float32)

    def pack_xt(xb):
        # [D, N] -> [P, 4, DT*512]
        return np.ascontiguousarray(
            xb.T.reshape(DT, P, 4, 512).transpose(1, 2, 0, 3).reshape(
                P, 4, DT * 512)).astype(bf)

    xts = [pack_xt(x[b]) for b in range(B)]
    w_augs = []
    wps = []
    for g in range(4):
        c0 = g * CPC
        wq = w_qkv[:, c0:c0 + CPC] * SCALE
        wk = w_qkv[:, D + c0:D + c0 + CPC]
        wv = w_qkv[:, 2 * D + c0:2 * D + c0 + CPC]
        blocks = [np.concatenate([wq[:, 0:128], wk[:, 0:128]], axis=1),
                  wv,
                  np.concatenate([wq[:, 128:256], wk[:, 128:256]], axis=1)]
        # each [D, 256] -> [P, DT, 256]; stack -> [P, 3, DT, 256]
        wb = np.stack([b.reshape(DT, P, 256).transpose(1, 0, 2)
                       for b in blocks], axis=1).reshape(P, 3, DT * 256)
        w_augs.append(np.ascontiguousarray(wb).astype(bf))
        wpp = w_proj[c0:c0 + CPC, :].reshape(2, P, D).transpose(1, 0, 2)
        wps.append(np.ascontiguousarray(wpp).astype(bf))

    in_maps = []
    for core in range(NCORES):
        b, g = core // 4, core % 4
        in_maps.append({"xt": xts[b], "w": w_augs[g], "wp": wps[g]})
    return in_maps


def _host_reference(x, w_qkv, b_qkv, w_proj, b_proj):
    """Exact numpy fallback (used only if b_q is nonzero, which the
    problem's setup_inputs never produces)."""
    x = np.asarray(x, np.float32)
    qkv = x @ np.asarray(w_qkv, np.float32) + np.asarray(b_qkv, np.float32)
    qkv = qkv.reshape(B, N, 3, H, HD).transpose(2, 0, 3, 1, 4)
    q, k, v = qkv[0], qkv[1], qkv[2]
    att = np.einsum("bhnd,bhmd->bhnm", q, k) * SCALE
    att = np.exp(att - att.max(-1, keepdims=True))
    att /= att.sum(-1, keepdims=True)
    out = np.einsum("bhnm,bhmd->bhnd", att, v)
    out = out.transpose(0, 2, 1, 3).reshape(B, N, D)
    return out @ np.asarray(w_proj, np.float32) + np.asarray(b_proj,
                                                             np.float32)


def core_reference(in_map):
    """Numpy reference for ONE core's shard (for CoreSim verification)."""
    xtp = np.asarray(in_map["xt"], np.float32).reshape(P, 4, DT, 512)
    wbp = np.asarray(in_map["w"], np.float32).reshape(P, 3, DT, 256)
    wpp = np.asarray(in_map["wp"], np.float32)  # [P, 2, D]
    xt = xtp.transpose(2, 0, 1, 3).reshape(D, N)
    wb = wbp.transpose(2, 0, 1, 3).reshape(D, 3, 256)
    wp = wpp.transpose(1, 0, 2).reshape(CPC, D)
    qkv = np.concatenate([xt.T @ wb[:, 0], xt.T @ wb[:, 2],
                          xt.T @ wb[:, 1]], axis=1)  # [N, q01k01 q23k23 v]
    out = np.zeros((N, CPC), np.float32)
    for h in range(HPC):
        pair, idx = h // 2, h % 2
        q = qkv[:, 256 * pair + idx * HD:256 * pair + (idx + 1) * HD]
        k = qkv[:, 256 * pair + 128 + idx * HD:
                256 * pair + 128 + (idx + 1) * HD]
        v = qkv[:, 2 * CPC + h * HD:2 * CPC + (h + 1) * HD]
        s = q @ k.T  # scale already folded into wq
        p = np.exp(s - s.max(axis=-1, keepdims=True))
        p /= p.sum(axis=-1, keepdims=True)
        out[:, h * HD:(h + 1) * HD] = p @ v
    return out @ wp  # [N, D] partial


def kernel(x, w_qkv, b_qkv, w_proj, b_proj):
    from concourse.bass_utils import run_bass_kernel_spmd

    b_qkv = np.asarray(b_qkv, np.float32)
    if np.any(b_qkv[:D]):
        # nonzero q-bias does not cancel in softmax; exact host fallback
        # (never taken for this problem's setup_inputs)
        return _host_reference(x, w_qkv, b_qkv, w_proj, b_proj)

    in_maps = make_in_maps(x, w_qkv, b_qkv, w_proj)
    if "nc" not in _CACHE:
        _CACHE["nc"] = build_nc()
    res = run_bass_kernel_spmd(_CACHE["nc"], in_maps,
                               core_ids=list(range(NCORES)))
    outs = [np.asarray(r["y"], np.float32) for r in res.results]
    y = np.empty((B, N, D), np.float32)
    for b in range(B):
        y[b] = outs[4 * b] + outs[4 * b + 1] + outs[4 * b + 2] + outs[4 * b + 3]
    # bias: b_k cancels in softmax; b_v shifts attention output by a
    # constant -> y += b_v @ w_proj; plus the projection bias
    y += b_qkv[2 * D:] @ np.asarray(w_proj, np.float32)
    y += np.asarray(b_proj, np.float32)
    return y


# revision 23
# speedup vs baseline: 1.0028x; 1.0028x over previous
"""Multi-head self-attention Trainium2 kernel.

Problem: B=2, N=2048, D=1024, H=16 heads (HD=64), fp32 I/O.

Sharding (8 cores): core c handles batch b = c//4 and the 4-head group
g = c%4 (data parallel on B, tensor parallel on heads).  Each core:
  1. QKV projection for its 768 columns (q cols pre-scaled by HD^-0.5),
     producing qT/kT channel-major and V row-major augmented with a
     ones column (softmax denominator rides the PV matmul as row 64).
  2. Transposed attention, two heads packed per pass (head A in PE rows
     0-63, head B in rows 64-127 -> concurrent row-group matmuls):
     S^T scores in PSUM, one exp per m-tile on ScalarE (no max
     subtraction -- logits are O(1) here), PV contracting over m.
  3. Normalization: reciprocal of the denominator row, broadcast via a
     K=1 matmul, multiply into outT.
  4. Output projection against its 256 rows of w_proj -> bf16 partial.
Host sums the 4 partials per batch and adds the bias terms.

Schedule: the kernel is jointly limited by ACT (128 exps of [128,1024],
~1.15us each, ~147us total) and the PE (~137us of matmul streams), so
the emission is paced to keep both near-busy from t~10us on:
  - DMA priority order (3 queues): pair-0 q/k weight cols + xt first
    half + v weights (ramp set, ~3MB), then xt second half, pair-1 q/k
    cols, w_proj.  The prologue computes exactly what quarter 0 needs
    first (kT m0-511, qT n0-511, V m0-255) paced by the arriving DMAs,
    so the first exp lands ~landmark 10-11us.
  - All remaining qT/kT column blocks, V groups, PV accumulations and
    output-projection blocks are emitted through a deadline/budget
    pacer: each is released only when the modeled PE backlog is below
    the modeled ACT backlog (so the in-order PE queue never delays the
    scores->exp chain), or when its deadline forces it.
  - PV runs up to 4 iterations behind exp (eb ring depth 6), which
    spreads quarter-0's forced V work into quarter 1 and carries each
    quarter's PV drain into the next quarter (no boundary stall).
PSUM (8 banks, all used): scores 2x[128,1024] (4), PV accum 2x[65,512]
(2), one [128,512] ring for qk/proj generations (1), one [128,512]
ring for V-pair generations + epilogue broadcasts (1).
"""

import numpy as np
import ml_dtypes

B, N, D, H = 2, 2048, 1024, 16
HD = D // H  # 64
SCALE = HD ** -0.5
NCORES = 8
HPC = H // 4  # heads per core
CPC = HPC * HD  # channels per core = 256
P = 128
DT = D // P  # 8 contraction tiles
NT = N // P  # 16 sequence tiles

_CACHE = {}

# pacer cost constants (ns, approximate)
C_SC = 230      # score pair (row-group concurrent)
C_EXP = 1150    # exp [128,1024]
C_PV = 450      # both heads' PV matmuls for one m-tile
C_QKC = 1000     # one 4-matmul qk chunk (half a [P,512] generation)
C_VG = 1800     # one packed V generation (2 m-tiles, 16 matmuls)
C_EPI = 450     # epilogue broadcast matmuls
C_PRJ = 560     # one [P,512] proj generation (2 matmuls)
SLACK = 700
PVLAG = 4       # max PV iterations behind exp (ebpool bufs - 2)


def build_nc():
    import concourse.tile as tile
    from concourse import bacc, mybir

    nc = bacc.Bacc("TRN2", target_bir_lowering=False, debug=False,
                   num_devices=NCORES)
    bf16 = mybir.dt.bfloat16
    xt = nc.dram_tensor("xt", [P, 4, DT * 512], bf16,
                        kind="ExternalInput").ap()
    w = nc.dram_tensor("w", [P, 3, DT * 256], bf16,
                       kind="ExternalInput").ap()
    wp = nc.dram_tensor("wp", [P, 2, D], bf16, kind="ExternalInput").ap()
    y = nc.dram_tensor("y", [N, D], bf16, kind="ExternalOutput").ap()

    with tile.TileContext(nc) as tc:
        _mha_tile_kernel(tc, y, xt, w, wp)
    nc.compile()
    return nc


def _mha_tile_kernel(tc, y, xt, w, wp):
    from contextlib import ExitStack
    from concourse import mybir

    nc = tc.nc
    bf16 = mybir.dt.bfloat16
    f32 = mybir.dt.float32
    EXP = mybir.ActivationFunctionType.Exp
    # qk_sb slot -> (w block, col offset); host w blocks [q01|k01], [v],
    # [q23|k23], each [P, DT, 256]
    QKCOL = {0: (0, 0), 2: (0, 128), 1: (2, 0), 3: (2, 128)}

    with ExitStack() as ctx:
        consts = ctx.enter_context(tc.tile_pool(name="consts", bufs=1))
        work = ctx.enter_context(tc.tile_pool(name="work", bufs=1))
        ebpool = ctx.enter_context(tc.tile_pool(name="eb", bufs=6))
        ypool = ctx.enter_context(tc.tile_pool(name="yp", bufs=4))
        rpool = ctx.enter_context(tc.tile_pool(name="rp", bufs=8))
        pvspool = ctx.enter_context(tc.tile_pool(name="pvs", bufs=6))
        ps_sc = ctx.enter_context(
            tc.tile_pool(name="ps_sc", bufs=2, space="PSUM"))   # 4 banks
        ps_pv = ctx.enter_context(
            tc.tile_pool(name="ps_pv", bufs=2, space="PSUM"))   # 2 banks
        ps_flex = ctx.enter_context(
            tc.tile_pool(name="ps_flex", bufs=2, space="PSUM"))  # 2 banks

        # ---- input DMA: few large strided transfers (dispatch costs
        # ~650ns/queue and each queue ring only allows 4 outstanding, so
        # many small DMAs serialize the ramp).  Priority: pair-0 q/k cols
        # + v cols + xt cols 0-511 (everything quarter 0 mt0-3 needs),
        # then xt 512-1023, xt second half, pair-1 q/k cols, w_proj.
        w_sb = work.tile([P, 3, DT * 256], bf16, tag="w")
        xt_sb = work.tile([P, 4, DT * 512], bf16, tag="xt")
        wp_sb = work.tile([P, 2, D], bf16, tag="wp")
        # Inputs are host-packed in SBUF layout (partition-major), so each
        # transfer is one DMA with multi-KB contiguous lines at full wire
        # rate.  Priority: pair-0 q/k cols, xt n0-511, v cols (the ramp
        # set, 2MB), xt n512-1023, xt second half, pair-1 q/k cols, wp.
        # every transfer is a plain 2D contiguous slice ([128 x <=4KB
        # lines]); 3D+ slice patterns both transfer slowly and mis-order
        # against their completion semaphores on hardware
        H2 = DT * 512 // 2
        # sweep-critical set first on all three queues (w qk01+v, xt block
        # 0): ~2MB of wire before the prologue can finish; everything else
        # strictly after
        nc.sync.dma_start(w_sb[:, 0], w[:, 0])            # q01|k01 cols
        nc.gpsimd.dma_start(xt_sb[:, 0, 0:H2], xt[:, 0, 0:H2])
        nc.scalar.dma_start(xt_sb[:, 0, H2:2 * H2], xt[:, 0, H2:2 * H2])
        nc.sync.dma_start(w_sb[:, 1], w[:, 1])            # v cols
        nc.gpsimd.dma_start(xt_sb[:, 1, 0:H2], xt[:, 1, 0:H2])
        nc.scalar.dma_start(xt_sb[:, 1, H2:2 * H2], xt[:, 1, H2:2 * H2])
        nc.gpsimd.dma_start(xt_sb[:, 2, 0:H2], xt[:, 2, 0:H2])
        nc.scalar.dma_start(xt_sb[:, 2, H2:2 * H2], xt[:, 2, H2:2 * H2])
        nc.gpsimd.dma_start(xt_sb[:, 3, 0:H2], xt[:, 3, 0:H2])
        nc.scalar.dma_start(xt_sb[:, 3, H2:2 * H2], xt[:, 3, H2:2 * H2])
        nc.sync.dma_start(w_sb[:, 2], w[:, 2])            # q23|k23 cols
        nc.sync.dma_start(wp_sb[:, 0], wp[:, 0])
        nc.sync.dma_start(wp_sb[:, 1], wp[:, 1])

        ones_sb = consts.tile([1, N], bf16, tag="ones")
        nc.vector.memset(ones_sb, 1.0)
        qk_sb = work.tile([P, 4, N], bf16, tag="qk")
        vaug_sb = work.tile([P, NT, HPC, HD + 1], bf16, tag="vaug")
        nc.vector.memset(vaug_sb[:, :, :, HD:HD + 1], 1.0)
        outT_sb = work.tile([P, 2, N], bf16, tag="outT")

        # warm the Exp table during the DMA ramp (one-time 1.3us load)
        wrm = consts.tile([1, 1], f32, tag="wrm")
        nc.vector.memset(wrm, 0.0)
        nc.scalar.activation(out=wrm, in_=wrm, func=EXP)

        # ---- pacer state ----
        pe_t = [0.0]
        act_t = [0.0]

        # ---- prologue: kT m0-511 / qT n0-511 / V m0-255, DMA-paced ----
        pro_k = ps_flex.tile([P, 512], f32, tag="flex", name="prok")
        pro_q = ps_sc.tile([P, 512], f32, tag="sc", name="proq")
        v01 = ps_flex.tile([P, 2, CPC], f32, tag="flex", name="v01")
        for dt in range(DT):
            st, sp = dt == 0, dt == DT - 1
            nc.tensor.matmul(
                pro_k, lhsT=w_sb[:, 0, dt * 256 + 128:dt * 256 + 256],
                rhs=xt_sb[:, 0, dt * 512:(dt + 1) * 512], start=st, stop=sp)
            nc.tensor.matmul(
                pro_q, lhsT=w_sb[:, 0, dt * 256:dt * 256 + 128],
                rhs=xt_sb[:, 0, dt * 512:(dt + 1) * 512], start=st, stop=sp)
            # v01[:, 0] is its own accumulation group in its bank; it may
            # interleave with the other banks' groups above but not with
            # the v01[:, 1] group (one pending group per bank), which is
            # deferred to a deadline-0 filler below.
            nc.tensor.matmul(
                v01[:, 0], lhsT=xt_sb[:, 0, dt * 512:dt * 512 + P],
                rhs=w_sb[:, 1, dt * 256:(dt + 1) * 256], start=st, stop=sp)
        nc.scalar.copy(out=qk_sb[:, 2, 0:512], in_=pro_k)
        nc.scalar.copy(out=qk_sb[:, 0, 0:512], in_=pro_q)

        def v01b():
            for dt in range(DT):
                nc.tensor.matmul(
                    v01[:, 1], lhsT=xt_sb[:, 0, dt * 512 + P:dt * 512 + 2 * P],
                    rhs=w_sb[:, 1, dt * 256:(dt + 1) * 256],
                    start=(dt == 0), stop=(dt == DT - 1))
            nc.vector.tensor_copy(
                out=vaug_sb[:, 0:2, :, 0:HD],
                in_=v01.rearrange("p two (h d) -> p two h d", h=HPC))

        # ---- qk generations ([P,512] psum on the qkf ring) ----
        def qk_gen_chunks(slot, j, copy_eng=None):
            state = {}

            def emit(dts, last):
                if "ps" not in state:
                    state["ps"] = ps_flex.tile([P, 512], f32, tag="flex",
                                              name=f"qk{slot}{j}")
                ps = state["ps"]
                blk, c0 = QKCOL[slot]
                for dt in dts:
                    nc.tensor.matmul(
                        ps, lhsT=w_sb[:, blk, dt * 256 + c0:dt * 256 + c0 + P],
                        rhs=xt_sb[:, j, dt * 512:(dt + 1) * 512],
                        start=(dt == 0), stop=(dt == DT - 1))
                if last:
                    dst = qk_sb[:, slot, j * 512:(j + 1) * 512]
                    if copy_eng is nc.scalar:
                        nc.scalar.copy(out=dst, in_=ps)
                    else:
                        nc.vector.tensor_copy(out=dst, in_=ps)

            return [lambda: emit(range(0, 4), False),
                    lambda: emit(range(4, DT), True)]

        # static fillers: (deadline git, cost, fn)
        fillers = []

        def add_gen(slot, j, d0, d1, copy_eng=None):
            a, b = qk_gen_chunks(slot, j, copy_eng)
            fillers.append([d0, C_QKC, a])
            fillers.append([d1, C_QKC, b])

        fillers.append([0, 950, v01b])   # V m128-255 (need before PV pops)
        add_gen(2, 1, 1, 2)      # kT m512-1023 (need git 4)
        add_gen(2, 2, 5, 6)      # kT m1024-1535 (need git 8)
        add_gen(2, 3, 9, 10)      # kT m1536-2047 (need git 12)
        add_gen(0, 1, 11, 13)    # qT n512-1023  (need git 16)
        add_gen(0, 2, 25, 27)    # qT n1024-1535 (need git 32)
        add_gen(0, 3, 41, 43)    # qT n1536-2047 (need git 48)
        add_gen(3, 0, 50, 52)    # kT p1 m0-511  (need git 64)
        add_gen(1, 0, 54, 56)    # qT p1 n0-511  (need git 64)
        add_gen(3, 1, 60, 62)    # kT p1 m512-1023 (need git 68)
        add_gen(3, 2, 64, 66)    # kT p1 m1024-1535 (need git 72)
        add_gen(3, 3, 70, 71)    # kT p1 m1536-2047 (need git 76)
        add_gen(1, 1, 74, 76)    # qT p1 n512-1023 (need git 80)
        add_gen(1, 2, 88, 90)    # qT p1 n1024-1535 (need git 96)
        add_gen(1, 3, 104, 106)  # qT p1 n1536-2047 (need git 112)
        fil_i = [0]

        # ---- V generations (packed 2 m-tiles, vps ring) ----
        v_next = [2]   # m-tiles 0,1 done in the prologue

        def emit_v_gen():
            mt = v_next[0]
            v_next[0] += 2
            ps = ps_flex.tile([P, 2, CPC], f32, tag="flex", name=f"v{mt}")
            for k2 in range(2):
                mtk = mt + k2
                o = (mtk % 4) * P
                for dt in range(DT):
                    nc.tensor.matmul(
                        ps[:, k2],
                        lhsT=xt_sb[:, mtk // 4, dt * 512 + o:dt * 512 + o + P],
                        rhs=w_sb[:, 1, dt * 256:(dt + 1) * 256],
                        start=(dt == 0), stop=(dt == DT - 1))
            nc.vector.tensor_copy(
                out=vaug_sb[:, mt:mt + 2, :, 0:HD],
                in_=ps.rearrange("p two (h d) -> p two h d", h=HPC))
            pe_t[0] += C_VG

        # ---- PV pipeline (lag-K behind exp), quarter finish ----
        from collections import deque
        pvq = deque()            # (git, pair, q, mt, eb)
        qpv = {}
        pending = []             # staged epilogue callables
        cur_git = [0]
        qfin = [-10]             # git of the last quarter-finish

        def finish_quarter(pair, q, pv):
            qfin[0] = cur_git[0]
            last = pair == 1 and q == 3
            pvs = [pvspool.tile([HD + 1, 512], f32, tag="pvs",
                                name=f"pvs{pair}{q}{i}") for i in range(2)]
            dcps = [rpool.tile([1, 512], f32, tag="dcp",
                               name=f"dcp{pair}{q}{i}") for i in range(2)]
            for i in range(2):
                # the final quarter splits across DVE+ACT for minimum tail
                # (GPSIMD cannot access PSUM, so DVE carries the rest);
                # the denominator row moves to partition 0 for the DVE
                # reciprocal ucode, which needs a partition-0 operand
                if last:
                    nc.scalar.copy(out=pvs[i], in_=pv[i])
                    nc.scalar.copy(out=dcps[i], in_=pv[i][HD:HD + 1, :])
                else:
                    nc.vector.tensor_copy(out=pvs[i], in_=pv[i])
                    nc.vector.tensor_copy(out=dcps[i],
                                          in_=pv[i][HD:HD + 1, :])
            rbfs = [rpool.tile([1, 512], bf16, tag="rbf",
                               name=f"rbf{pair}{q}{i}") for i in range(2)]

            def recs():
                for i in range(2):
                    rec = rpool.tile([1, 512], f32, tag="rec",
                                     name=f"rec{pair}{q}{i}")
                    nc.vector.reciprocal_approx_fast(out=rec, in_=dcps[i])
                    nc.vector.tensor_copy(out=rbfs[i], in_=rec)

            def epi():
                n0 = q * 512
                for i in range(2):
                    bp = i * HD
                    bc = ps_flex.tile([HD, 512], f32, tag="flex",
                                     name=f"bc{pair}{q}{i}")
                    nc.tensor.matmul(bc, lhsT=ones_sb[:, 0:HD], rhs=rbfs[i],
                                     start=True, stop=True)
                    nc.vector.tensor_mul(
                        out=outT_sb[bp:bp + HD, pair, n0:n0 + 512],
                        in0=bc, in1=pvs[i][0:HD, :])
                pe_t[0] += C_EPI
                if pair == 1 and q < 3:
                    for nt in range(4 * q, 4 * q + 4):
                        add_proj(nt)

            pending.append(recs)
            pending.append(epi)

        def pop_pv():
            _, pair, q, mt, eb = pvq.popleft()
            if pair == 0 and v_next[0] <= mt:
                emit_v_gen()
            if mt == 0:
                qpv[(pair, q)] = [
                    ps_pv.tile([HD + 1, 512], f32, tag="pv",
                               name=f"pv{pair}{q}{i}") for i in range(2)]
            pv = qpv[(pair, q)]
            for i in range(2):
                nc.tensor.matmul(pv[i],
                                 lhsT=vaug_sb[:, mt, 2 * pair + i, :],
                                 rhs=eb[:, i * 512:(i + 1) * 512],
                                 start=(mt == 0), stop=(mt == NT - 1))
            pe_t[0] += C_PV
            if mt == NT - 1:
                finish_quarter(pair, q, pv)

        # ---- output projection generations (qkf ring) ----
        in_tail = [False]

        def add_proj(nt):
            yt = ypool.tile([P, D], bf16, tag="y", name=f"y{nt}")

            def half(hf):
                ps = ps_flex.tile([P, 512], f32, tag="flex", name=f"pj{nt}{hf}")
                for ct in range(2):
                    nc.tensor.matmul(
                        ps, lhsT=outT_sb[:, ct, nt * P:(nt + 1) * P],
                        rhs=wp_sb[:, ct, hf * 512:(hf + 1) * 512],
                        start=(ct == 0), stop=(ct == 1))
                # post-loop, ACT is idle: split the psum release with DVE
                ce = nc.scalar if (in_tail[0] and hf == 0) else nc.vector
                if ce is nc.scalar:
                    ce.copy(out=yt[:, hf * 512:(hf + 1) * 512], in_=ps)
                else:
                    ce.tensor_copy(out=yt[:, hf * 512:(hf + 1) * 512], in_=ps)
                if hf == 1:
                    eng = nc.sync if nt % 2 == 0 else nc.gpsimd
                    eng.dma_start(y[nt * P:(nt + 1) * P, :], yt)

            fillers.append([995, C_PRJ, lambda: half(0)])
            fillers.append([996, C_PRJ, lambda: half(1)])

        # ---- main attention loop ----
        for pair in range(2):
            for q in range(4):
                n0 = q * 512
                for mt in range(NT):
                    git = pair * 64 + q * 16 + mt
                    cur_git[0] = git
                    ps = ps_sc.tile([P, 1024], f32, tag="sc")
                    for i in range(2):
                        bp = i * HD
                        nc.tensor.matmul(
                            ps[:, i * 512:(i + 1) * 512],
                            lhsT=qk_sb[bp:bp + HD, 2 + pair,
                                       mt * P:(mt + 1) * P],
                            rhs=qk_sb[bp:bp + HD, pair, n0:n0 + 512],
                            start=True, stop=True)
                    pe_t[0] += C_SC
                    eb = ebpool.tile([P, 1024], bf16, tag="eb")
                    nc.scalar.activation(out=eb, in_=ps, func=EXP)
                    act_t[0] += C_EXP
                    pvq.append((git, pair, q, mt, eb))
                    # forced: PV lag cap, filler deadlines; at the very
                    # end run PV nearly caught-up so the tail chain is short
                    while len(pvq) > (PVLAG if git < 124 else 1):
                        pop_pv()
                    while fil_i[0] < len(fillers) and \
                            fillers[fil_i[0]][0] <= git:
                        c = fillers[fil_i[0]]
                        fil_i[0] += 1
                        c[2]()
                        pe_t[0] += c[1]
                    if pending and mt % 2 == 0:
                        pending.pop(0)()
                    # budget: release work while the PE backlog trails ACT
                    # (max 2 filler generations per iteration to avoid psum
                    # ring bursts; PV pops need a 2-iteration-old exp and 2
                    # iterations of spacing after a quarter release)
                    nfil = 0
                    while pe_t[0] < act_t[0] - SLACK:
                        if nfil < 1 and fil_i[0] < len(fillers) and \
                                fillers[fil_i[0]][0] <= git + PVLAG:
                            c = fillers[fil_i[0]]
                            fil_i[0] += 1
                            c[2]()
                            pe_t[0] += c[1]
                            nfil += 1
                        elif pvq and git - pvq[0][0] >= 2 and \
                                git >= qfin[0] + 2:
                            pop_pv()
                        elif nfil < 1 and fil_i[0] < len(fillers):
                            c = fillers[fil_i[0]]
                            fil_i[0] += 1
                            c[2]()
                            pe_t[0] += c[1]
                            nfil += 1
                        else:
                            break

        # ---- tail: drain PV pipeline, last epilogue, last projections ----
        in_tail[0] = True
        while pvq:
            pop_pv()
        for fn in pending:
            fn()
        while fil_i[0] < len(fillers):
            c = fillers[fil_i[0]]
            fil_i[0] += 1
            c[2]()
        for nt in range(12, 16):
            ps = ps_sc.tile([P, 1024], f32, tag="sc", name=f"pjt{nt}")
            for ct in range(2):
                for ec in range(2):
                    nc.tensor.matmul(
                        ps[:, ec * 512:(ec + 1) * 512],
                        lhsT=outT_sb[:, ct, nt * P:(nt + 1) * P],
                        rhs=wp_sb[:, ct, ec * 512:(ec + 1) * 512],
                        start=(ct == 0), stop=(ct == 1))
            yt = ypool.tile([P, D], bf16, tag="y", name=f"yt{nt}")
            nc.scalar.copy(out=yt[:, 0:512], in_=ps[:, 0:512])
            nc.vector.tensor_copy(out=yt[:, 512:1024], in_=ps[:, 512:1024])
            eng = (nc.sync, nc.gpsimd, nc.scalar, nc.sync)[nt - 12]
            eng.dma_start(y[nt * P:(nt + 1) * P, :], yt)


def make_in_maps(x, w_qkv, b_qkv, w_proj):
    """Build the 8 per-core input dicts (host-side sharding).

    Biases are not sent to the device: b_k shifts every logit in a
    softmax row by the same amount (cancels exactly), b_v shifts the
    attention output by a constant (folded into y on the host as
    b_v @ w_proj), and b_q is zero for this problem (kernel() falls
    back to an exact host path if it ever is not).

    All inputs are packed in the exact SBUF layout (partition-major) so
    each tensor loads as one or few DMAs with multi-KB contiguous lines:
    xt [P, 4 n-blocks, DT, 512], w [P, 3 blocks ([q01|k01], [v],
    [q23|k23]), DT, 256], wp [P, 2, D].
    """
    bf = ml_dtypes.bfloat16
    x = np.asarray(x, np.float32)
    w_qkv = np.asarray(w_qkv, np.float32)
    w_proj = np.asarray(w_proj, np.float32)

    def pack_xt(xb):
        # [D, N] -> [P, 4, DT*512]
        return np.ascontiguousarray(
            xb.T.reshape(DT, P, 4, 512).transpose(1, 2, 0, 3).reshape(
                P, 4, DT * 512)).astype(bf)

    xts = [pack_xt(x[b]) for b in range(B)]
    w_augs = []
    wps = []
    for g in range(4):
        c0 = g * CPC
        wq = w_qkv[:, c0:c0 + CPC] * SCALE
        wk = w_qkv[:, D + c0:D + c0 + CPC]
        wv = w_qkv[:, 2 * D + c0:2 * D + c0 + CPC]
        blocks = [np.concatenate([wq[:, 0:128], wk[:, 0:128]], axis=1),
                  wv,
                  np.concatenate([wq[:, 128:256], wk[:, 128:256]], axis=1)]
        # each [D, 256] -> [P, DT, 256]; stack -> [P, 3, DT, 256]
        wb = np.stack([b.reshape(DT, P, 256).transpose(1, 0, 2)
                       for b in blocks], axis=1).reshape(P, 3, DT * 256)
        w_augs.append(np.ascontiguousarray(wb).astype(bf))
        wpp = w_proj[c0:c0 + CPC, :].reshape(2, P, D).transpose(1, 0, 2)
        wps.append(np.ascontiguousarray(wpp).astype(bf))

    in_maps = []
    for core in range(NCORES):
        b, g = core // 4, core % 4
        in_maps.append({"xt": xts[b], "w": w_augs[g], "wp": wps[g]})
    return in_maps


def _host_reference(x, w_qkv, b_qkv, w_proj, b_proj):
    """Exact numpy fallback (used only if b_q is nonzero, which the
    problem's setup_inputs never produces)."""
    x = np.asarray(x, np.float32)
    qkv = x @ np.asarray(w_qkv, np.float32) + np.asarray(b_qkv, np.float32)
    qkv = qkv.reshape(B, N, 3, H, HD).transpose(2, 0, 3, 1, 4)
    q, k, v = qkv[0], qkv[1], qkv[2]
    att = np.einsum("bhnd,bhmd->bhnm", q, k) * SCALE
    att = np.exp(att - att.max(-1, keepdims=True))
    att /= att.sum(-1, keepdims=True)
    out = np.einsum("bhnm,bhmd->bhnd", att, v)
    out = out.transpose(0, 2, 1, 3).reshape(B, N, D)
    return out @ np.asarray(w_proj, np.float32) + np.asarray(b_proj,
                                                             np.float32)


def core_reference(in_map):
    """Numpy reference for ONE core's shard (for CoreSim verification)."""
    xtp = np.asarray(in_map["xt"], np.float32).reshape(P, 4, DT, 512)
    wbp = np.asarray(in_map["w"], np.float32).reshape(P, 3, DT, 256)
    wpp = np.asarray(in_map["wp"], np.float32)  # [P, 2, D]
    xt = xtp.transpose(2, 0, 1, 3).reshape(D, N)
    wb = wbp.transpose(2, 0, 1, 3).reshape(D, 3, 256)
    wp = wpp.transpose(1, 0, 2).reshape(CPC, D)
    qkv = np.concatenate([xt.T @ wb[:, 0], xt.T @ wb[:, 2],
                          xt.T @ wb[:, 1]], axis=1)  # [N, q01k01 q23k23 v]
    out = np.zeros((N, CPC), np.float32)
    for h in range(HPC):
        pair, idx = h // 2, h % 2
        q = qkv[:, 256 * pair + idx * HD:256 * pair + (idx + 1) * HD]
        k = qkv[:, 256 * pair + 128 + idx * HD:
                256 * pair + 128 + (idx + 1) * HD]
        v = qkv[:, 2 * CPC + h * HD:2 * CPC + (h + 1) * HD]
        s = q @ k.T  # scale already folded into wq
        p = np.exp(s - s.max(axis=-1, keepdims=True))
        p /= p.sum(axis=-1, keepdims=True)
        out[:, h * HD:(h + 1) * HD] = p @ v
    return out @ wp  # [N, D] partial


def kernel(x, w_qkv, b_qkv, w_proj, b_proj):
    from concourse.bass_utils import run_bass_kernel_spmd

    b_qkv = np.asarray(b_qkv, np.float32)
    if np.any(b_qkv[:D]):
        # nonzero q-bias does not cancel in softmax; exact host fallback
        # (never taken for this problem's setup_inputs)
        return _host_reference(x, w_qkv, b_qkv, w_proj, b_proj)

    in_maps = make_in_maps(x, w_qkv, b_qkv, w_proj)
    if "nc" not in _CACHE:
        _CACHE["nc"] = build_nc()
    res = run_bass_kernel_spmd(_CACHE["nc"], in_maps,
                               core_ids=list(range(NCORES)))
    outs = [np.asarray(r["y"], np.float32) for r in res.results]
    y = np.empty((B, N, D), np.float32)
    for b in range(B):
        y[b] = outs[4 * b] + outs[4 * b + 1] + outs[4 * b + 2] + outs[4 * b + 3]
    # bias: b_k cancels in softmax; b_v shifts attention output by a
    # constant -> y += b_v @ w_proj; plus the projection bias
    y += b_qkv[2 * D:] @ np.asarray(w_proj, np.float32)
    y += np.asarray(b_proj, np.float32)
    return y
